# revision 8
# baseline (speedup 1.0000x reference)
"""MoE routing kernel (nn_JSMLP): per-row expert-indexed 3-layer MLP.

  out[n] = Wl[i] @ tanh(W2[i] @ tanh(W1[i] @ x[n] + b1[i]) + b2[i]) + bl[i],  i = ind[n]

Sharding strategy (hardcoded): expert-parallel across the 8 cores.
Host groups rows by expert (argsort of ind), pads each expert's rows to a
uniform capacity C, and assigns 32 consecutive experts to each core. Each
core then runs dense per-expert GEMMs in a transposed layout (hidden dim on
SBUF partitions, rows on the free dim), in bf16 with fp32 PSUM accumulation:

  L1: H1ᵀ[256, C] = W1augᵀ.T @ xaugᵀ      (bias via ones-row augmentation, K=65)
  L2: H2ᵀ[256, C] = W2ᵀ.T @ tanh(H1ᵀ)     (K=256 split in 2, bias via a tiny
                                           block-diagonal ones matmul)
  L3: outᵀ[64, C] = Wlᵀ.T @ tanh(H2ᵀ)     (two experts packed per 128-partition
                                           PSUM tile, bias as in L2)

tanh runs on ScalarE over wide multi-expert spans to amortize the per-op
overhead. Everything is statically compiled after inspecting the routing
(capacity C is derived from the actual max expert load), so the single SPMD
program is identical across cores and only the per-core data differs.
"""

import numpy as np
import ml_dtypes

N, IN_DIM, H1, H2, LIN, NEXP = 16384, 64, 256, 256, 64, 256
NCORES = 8
EPC = NEXP // NCORES  # experts per core

BF16 = ml_dtypes.bfloat16

_cache = {}


def _build_program(C, B):
    """Build the SPMD Bass program for capacity C with B experts per block."""
    import concourse.bass as bass
    import concourse.tile as tile
    from concourse import bacc, mybir

    S = B * C            # columns of one block's expert group
    ncb = B // 2         # column blocks in the packed L3 output
    blocks = EPC // B
    f32 = mybir.dt.float32
    bf16 = mybir.dt.bfloat16
    Tanh = mybir.ActivationFunctionType.Tanh

    nc = bacc.Bacc("TRN2", target_bir_lowering=False, debug=False,
                   num_devices=NCORES)

    # DMA count is the scarce resource (HWDGE descriptor generation is a
    # shared serial device, ~625ns per dma_start), so tensors are merged into
    # one [65, .]-partition load (x + W1) and one [128, .] load (W2/Wl
    # K-chunks) per block, plus a single constants load.
    XW = B * (C + 256)               # xg cols [0, B*C), w1t cols [B*C, XW)
    WB = B * 640                     # w2a | w2b | wla | wlb sections
    O_W2B, O_WLA, O_WLB = B * 256, 2 * B * 256, 2 * B * 256 + B * 64
    CT = blocks * 256 + blocks * 128 + S   # w2c | wlc | bdiag sections
    O_WLC, O_BD = blocks * 256, blocks * 256 + blocks * 128

    xw_d = nc.dram_tensor("xw", [blocks, 65, XW], bf16, kind="ExternalInput")
    wb_d = nc.dram_tensor("wb", [blocks, 128, WB], bf16, kind="ExternalInput")
    ct_d = nc.dram_tensor("ct", [B, CT], bf16, kind="ExternalInput")
    out_d = nc.dram_tensor("out", [128, blocks * ncb * C], bf16,
                           kind="ExternalOutput")

    with tile.TileContext(nc) as tc:
        with (
            tc.tile_pool(name="consts", bufs=1) as cpool,
            tc.tile_pool(name="wts", bufs=8) as wpool,
            tc.tile_pool(name="acts", bufs=8) as hpool,
            tc.tile_pool(name="ostage", bufs=1) as opool,
            tc.tile_pool(name="ph", bufs=2, space=bass.MemorySpace.PSUM) as phpool,
            tc.tile_pool(name="po", bufs=4, space=bass.MemorySpace.PSUM) as popool,
        ):
            ct = cpool.tile([B, CT], bf16, tag="ct")
            w2ct = ct[:, 0:O_WLC]
            wlct = ct[0:ncb, O_WLC:O_BD]
            bdt = ct[:, O_BD:O_BD + S]
            # bf16 staging/store: halves output bytes on the serial DMA pipe;
            # the DVE copy below does the fp32->bf16 cast for free
            ostage = opool.tile([128, blocks * ncb * C], bf16, tag="out")

            # Issue every input load up front on the SP queue, in dependency
            # order (xw before w2 before wl per block).  Splitting wb into its
            # w2/wl sections lets L2(b) start before wl(b) has landed, and the
            # last block's L2 overlap the tail of the stream.
            xwts, w2ts, wlts = [], [], []
            for b in range(blocks):
                xwt = wpool.tile([65, XW], bf16, tag="xw")
                nc.sync.dma_start(xwt[:], xw_d.ap()[b])
                w2t = wpool.tile([128, O_WLA], bf16, tag="w2")
                nc.sync.dma_start(w2t[:], wb_d.ap()[b][:, 0:O_WLA])
                wlt = wpool.tile([128, WB - O_WLA], bf16, tag="wl")
                nc.sync.dma_start(wlt[:], wb_d.ap()[b][:, O_WLA:WB])
                if b == 0:
                    nc.sync.dma_start(ct[:], ct_d.ap())
                xwts.append(xwt); w2ts.append(w2t); wlts.append(wlt)

            # PE executes matmuls strictly in program order, so the emission
            # order below software-pipelines the blocks: iteration b emits
            # L1(b), L2(b-1), L3(b-2).  PE then never sits in a tanh round
            # trip — it always has the next block's independent work queued —
            # and the pipeline keeps pace with the DMA stream.
            h1s, h2s, ph2s, pos = {}, {}, {}, {}

            def stage_l1(b):
                xwt = xwts[b]
                xgt = xwt[:, 0:B * C]
                w1tt = xwt[:, B * C:XW]
                # L1: H1preT[256, S] — hidden half t lives at cols
                # [t*512, t*512+S) (512-aligned so no matmul output crosses a
                # PSUM bank boundary).
                ph1 = phpool.tile([128, 1024], f32, tag="ph", name=f"ph1_{b}")
                for j in range(B):
                    for t in range(2):
                        nc.tensor.matmul(
                            ph1[:, t * 512 + j * C : t * 512 + (j + 1) * C],
                            w1tt[:, j * 256 + t * 128 : j * 256 + (t + 1) * 128],
                            xgt[:, j * C : (j + 1) * C],
                        )
                h1 = hpool.tile([128, 2 * S], bf16, tag="h1", name=f"h1_{b}")
                nc.scalar.activation(
                    h1[:].rearrange("p (t s) -> p t s", t=2),
                    ph1[:].rearrange("p (t s) -> p t s", t=2)[:, :, 0:S],
                    Tanh,
                )
                h1s[b] = h1

            def stage_l2(b):
                # L2: bias seeded over the chunk span, then 2 accumulating
                # K-chunks per expert.  The last block is split into expert
                # pairs so its first tanh can start before its last matmul.
                w2t, h1 = w2ts[b], h1s[b]
                w2at = w2t[:, 0:O_W2B]
                w2bt = w2t[:, O_W2B:O_WLA]
                chunks = [(0, B)] if b < blocks - 1 else [(0, 2), (2, B)]
                ph2 = phpool.tile([128, 1024], f32, tag="ph", name=f"ph2_{b}")
                h2 = hpool.tile([128, 2 * S], bf16, tag="h2", name=f"h2_{b}")
                for (j0, j1) in chunks:
                    # bias matmul LAST: every matmul in the group is then
                    # data-gated on h1/w2, so the Tile scheduler cannot hoist
                    # an always-ready bias matmul into the PE wait queue where
                    # its PSUM-slot WAR blocks the whole engine
                    for t in range(2):
                        for j in range(j0, j1):
                            nc.tensor.matmul(
                                ph2[:, t * 512 + j * C : t * 512 + (j + 1) * C],
                                w2at[:, j * 256 + t * 128 : j * 256 + (t + 1) * 128],
                                h1[:, j * C : (j + 1) * C],
                                start=True, stop=False, skip_group_check=True,
                            )
                            nc.tensor.matmul(
                                ph2[:, t * 512 + j * C : t * 512 + (j + 1) * C],
                                w2bt[:, j * 256 + t * 128 : j * 256 + (t + 1) * 128],
                                h1[:, S + j * C : S + (j + 1) * C],
                                start=False, stop=False, skip_group_check=True,
                            )
                        nc.tensor.matmul(
                            ph2[:, t * 512 + j0 * C : t * 512 + j1 * C],
                            w2ct[:, (b * 2 + t) * 128 : (b * 2 + t + 1) * 128],
                            bdt[:, j0 * C : j1 * C],
                            start=False, stop=True, skip_group_check=True,
                        )
                    nc.scalar.activation(
                        h2[:].rearrange("p (t s) -> p t s", t=2)[:, :, j0 * C : j1 * C],
                        ph2[:].rearrange("p (t s) -> p t s", t=2)[:, :, j0 * C : j1 * C],
                        Tanh,
                    )
                ph2s[b], h2s[b] = ph2, h2

            def stage_l3(b):
                # L3: experts packed 2-per-partition-block: expert j -> output
                # partitions [64*(j%2), +64), columns [(j//2)*C, +C).
                wlt, h2 = wlts[b], h2s[b]
                wlat = wlt[:, 0:O_WLB - O_WLA]
                wlbt = wlt[:, O_WLB - O_WLA:WB - O_WLA]
                po = popool.tile([128, ncb * C], f32, tag="po", name=f"po_{b}")
                for j in range(B):
                    h_, cb = j % 2, j // 2
                    nc.tensor.matmul(
                        po[h_ * 64 : (h_ + 1) * 64, cb * C : (cb + 1) * C],
                        wlat[:, j * 64 : (j + 1) * 64],
                        h2[:, j * C : (j + 1) * C],
                        start=True, stop=False, skip_group_check=True,
                    )
                    nc.tensor.matmul(
                        po[h_ * 64 : (h_ + 1) * 64, cb * C : (cb + 1) * C],
                        wlbt[:, j * 64 : (j + 1) * 64],
                        h2[:, S + j * C : S + (j + 1) * C],
                        start=False, stop=False, skip_group_check=True,
                    )
                # bias last (see L2): keeps these matmuls out of the wait
                # queue until the data path is nearly done
                for h in range(2):
                    nc.tensor.matmul(
                        po[h * 64 : (h + 1) * 64, :],
                        wlct[:, (b * 2 + h) * 64 : (b * 2 + h + 1) * 64],
                        bdt[0:ncb, 0 : ncb * C],
                        start=False, stop=True, skip_group_check=True,
                    )
                nc.vector.tensor_copy(
                    ostage[:, b * ncb * C : (b + 1) * ncb * C], po[:]
                )
                if b % 2 == 1:
                    # early stores go via SWDGE on the otherwise-idle Pool
                    # engine (no shared-HWDGE pressure, and the ACT queue never
                    # stalls behind a compute-gated store); the final store
                    # uses the ACT HWDGE queue, whose gen latency (632+784)
                    # beats SWDGE's (1037+650) on the critical tail
                    q = nc.scalar if b == blocks - 1 else nc.gpsimd
                    q.dma_start(
                        out_d.ap()[:, (b - 1) * ncb * C : (b + 1) * ncb * C],
                        ostage[:, (b - 1) * ncb * C : (b + 1) * ncb * C],
                    )

            for b in range(blocks + 2):
                if b < blocks:
                    stage_l1(b)
                if 1 <= b <= blocks:
                    stage_l2(b - 1)
                if b >= 2:
                    stage_l3(b - 2)

    nc.compile()
    return nc


def _prep_inputs(x, ind, W1, b1, W2, b2, Wl, bl, C, B):
    """Group rows by expert and build the per-core padded device arrays."""
    blocks = EPC // B
    ncb = B // 2
    S = B * C

    order = np.argsort(ind, kind="stable")
    counts = np.bincount(ind, minlength=NEXP)
    offs = np.zeros(NEXP + 1, np.int64)
    np.cumsum(counts, out=offs[1:])
    rows = [order[offs[e]:offs[e + 1]] for e in range(NEXP)]

    # Augmented, transposed weight tables (built once across all cores).
    # W1augT[e] = [65, 256]: rows 0:64 = W1[e].T, row 64 = b1[e].
    w1aug = np.concatenate([W1, b1[:, :, None]], axis=2)  # [E, 256, 65]
    w2aug = np.concatenate([W2, b2[:, :, None]], axis=2)  # [E, 256, 257]
    wlaug = np.concatenate([Wl, bl[:, :, None]], axis=2)  # [E, 64, 257]

    XW = B * (C + 256)
    WB = B * 640
    O_W2B, O_WLA, O_WLB = B * 256, 2 * B * 256, 2 * B * 256 + B * 64
    CT = blocks * 256 + blocks * 128 + S
    O_WLC, O_BD = blocks * 256, blocks * 256 + blocks * 128

    in_maps = []
    for k in range(NCORES):
        es = np.arange(k * EPC, (k + 1) * EPC)
        xw = np.zeros((blocks, 65, XW), np.float32)
        xw[:, 64, 0:B * C] = 1.0  # ones row of the augmented x
        wb = np.empty((blocks, 128, WB), np.float32)
        ct = np.zeros((B, CT), np.float32)
        for b in range(blocks):
            for j in range(B):
                e = es[b * B + j]
                r = rows[e]
                xw[b, 0:64, j * C : j * C + len(r)] = x[r].T
                xw[b, :, B * C + j * 256 : B * C + (j + 1) * 256] = w1aug[e].T
                wb[b, :, j * 256 : (j + 1) * 256] = w2aug[e, :, 0:128].T
                wb[b, :, O_W2B + j * 256 : O_W2B + (j + 1) * 256] = \
                    w2aug[e, :, 128:256].T
                wb[b, :, O_WLA + j * 64 : O_WLA + (j + 1) * 64] = \
                    wlaug[e, :, 0:128].T
                wb[b, :, O_WLB + j * 64 : O_WLB + (j + 1) * 64] = \
                    wlaug[e, :, 128:256].T
                # w2c[j, b, t] = b2-augmented row of expert e, chunk t
                ct[j, b * 256 : (b + 1) * 256] = w2aug[e, :, 256]
            # wlc[cb, b, h] = bl-augmented row of expert B*b + 2*cb + h
            for cb in range(ncb):
                for h in range(2):
                    e = es[b * B + 2 * cb + h]
                    ct[cb, O_WLC + b * 128 + h * 64 : O_WLC + b * 128 + (h + 1) * 64] = \
                        wlaug[e, :, 256]
        for j in range(B):
            ct[j, O_BD + j * C : O_BD + (j + 1) * C] = 1.0
        in_maps.append({
            "xw": xw.astype(BF16),
            "wb": wb.astype(BF16),
            "ct": ct.astype(BF16),
        })
    return in_maps, rows


def _unscatter(results, rows, C, B):
    blocks = EPC // B
    ncb = B // 2
    out = np.empty((N, LIN), np.float32)
    for k in range(NCORES):
        arr = np.asarray(results[k]["out"], np.float32).reshape(2, 64, blocks, ncb, C)
        for b in range(blocks):
            for cb in range(ncb):
                for h in range(2):
                    e = k * EPC + b * B + 2 * cb + h
                    r = rows[e]
                    out[r, :] = arr[h, :, b, cb, 0:len(r)].T
    return out


def kernel(x, ind, W1, b1, W2, b2, Wl, bl):
    from concourse.bass_utils import run_bass_kernel_spmd

    x = np.asarray(x, np.float32)
    ind = np.asarray(ind).astype(np.int64)
    W1 = np.asarray(W1, np.float32); b1 = np.asarray(b1, np.float32)
    W2 = np.asarray(W2, np.float32); b2 = np.asarray(b2, np.float32)
    Wl = np.asarray(Wl, np.float32); bl = np.asarray(bl, np.float32)

    counts = np.bincount(ind, minlength=NEXP)
    C = max(32, int(np.ceil(counts.max() / 32)) * 32)
    assert C <= 256, f"expert load {counts.max()} too imbalanced for this kernel"
    B = 4 if C <= 128 else 2  # keep B*C <= 512 (one PSUM bank per block span)

    key = (C, B)
    if key not in _cache:
        _cache[key] = _build_program(C, B)
    nc = _cache[key]

    in_maps, rows = _prep_inputs(x, ind, W1, b1, W2, b2, Wl, bl, C, B)
    res = run_bass_kernel_spmd(nc, in_maps, core_ids=list(range(NCORES)))
    return _unscatter(res.results, rows, C, B)



# revision 11
# speedup vs baseline: 1.2135x; 1.2135x over previous
"""MoE routing kernel (nn_JSMLP): per-row expert-indexed 3-layer MLP.

  out[n] = Wl[i] @ tanh(W2[i] @ tanh(W1[i] @ x[n] + b1[i]) + b2[i]) + bl[i],  i = ind[n]

Sharding strategy (hardcoded): expert-parallel across the 8 cores.
Host groups rows by expert (argsort of ind), pads each expert's rows to a
uniform capacity C, and assigns 32 consecutive experts to each core. Each
core then runs dense per-expert GEMMs in a transposed layout (hidden dim on
SBUF partitions, rows on the free dim), in bf16 with fp32 PSUM accumulation:

  L1: H1ᵀ[256, C] = W1augᵀ.T @ xaugᵀ      (bias via ones-row augmentation, K=65)
  L2: H2ᵀ[256, C] = W2ᵀ.T @ tanh(H1ᵀ)     (K=256 split in 2, bias via a tiny
                                           block-diagonal ones matmul)
  L3: outᵀ[64, C] = Wlᵀ.T @ tanh(H2ᵀ)     (two experts packed per 128-partition
                                           PSUM tile, bias as in L2)

tanh runs on ScalarE over wide multi-expert spans to amortize the per-op
overhead. Everything is statically compiled after inspecting the routing
(capacity C is derived from the actual max expert load), so the single SPMD
program is identical across cores and only the per-core data differs.
"""

import numpy as np
import ml_dtypes

N, IN_DIM, H1, H2, LIN, NEXP = 16384, 64, 256, 256, 64, 256
NCORES = 8
EPC = NEXP // NCORES  # experts per core

BF16 = ml_dtypes.bfloat16

_cache = {}


def _build_program(C, B):
    """Build the SPMD Bass program for capacity C with B experts per block."""
    import concourse.bass as bass
    import concourse.tile as tile
    from concourse import bacc, mybir

    S = B * C            # columns of one block's expert group
    ncb = B // 2         # column blocks in the packed L3 output
    blocks = EPC // B
    f32 = mybir.dt.float32
    bf16 = mybir.dt.bfloat16
    Tanh = mybir.ActivationFunctionType.Tanh

    nc = bacc.Bacc("TRN2", target_bir_lowering=False, debug=False,
                   num_devices=NCORES)

    # DMA count is the scarce resource (HWDGE descriptor generation is a
    # shared serial device, ~625ns per dma_start), so tensors are merged into
    # one [65, .]-partition load (x + W1) and one [128, .] load (W2/Wl
    # K-chunks) per block, plus a single constants load.
    XW = B * (C + 256)               # xg cols [0, B*C), w1t cols [B*C, XW)
    WB = B * 640                     # w2a | w2b | wla | wlb sections
    O_W2B, O_WLA, O_WLB = B * 256, 2 * B * 256, 2 * B * 256 + B * 64
    CT = blocks * 256 + blocks * 128 + S   # w2c | wlc | bdiag sections
    O_WLC, O_BD = blocks * 256, blocks * 256 + blocks * 128

    xw_d = nc.dram_tensor("xw", [blocks, 65, XW], bf16, kind="ExternalInput")
    wb_d = nc.dram_tensor("wb", [blocks, 128, WB], bf16, kind="ExternalInput")
    ct_d = nc.dram_tensor("ct", [B, CT], bf16, kind="ExternalInput")
    out_d = nc.dram_tensor("out", [128, blocks * ncb * C], bf16,
                           kind="ExternalOutput")

    with tile.TileContext(nc) as tc:
        with (
            tc.tile_pool(name="consts", bufs=1) as cpool,
            tc.tile_pool(name="wts", bufs=8) as wpool,
            tc.tile_pool(name="acts", bufs=8) as hpool,
            tc.tile_pool(name="ostage", bufs=1) as opool,
            tc.tile_pool(name="ph", bufs=2, space=bass.MemorySpace.PSUM) as phpool,
            tc.tile_pool(name="po", bufs=4, space=bass.MemorySpace.PSUM) as popool,
        ):
            ct = cpool.tile([B, CT], bf16, tag="ct")
            w2ct = ct[:, 0:O_WLC]
            wlct = ct[0:ncb, O_WLC:O_BD]
            bdt = ct[:, O_BD:O_BD + S]
            # bf16 staging/store: halves output bytes on the serial DMA pipe;
            # the DVE copy below does the fp32->bf16 cast for free
            ostage = opool.tile([128, blocks * ncb * C], bf16, tag="out")

            # Issue every input load up front on the SP queue.  Stream order:
            # per-block [xw, w2] pairs first, then ALL wl sections at the end.
            # The L1/tanh/L2/tanh ring is then the only mid-stream compute
            # (cadence-feasible), the L3 epilogue is arrival-gated by the wl
            # tail of the stream, and the serial chain after the last input
            # byte is just L3(7)->copy->store instead of the full
            # L2->tanh->L3->copy->store.
            xwts, w2ts, wlts = [], [], []
            for b in range(blocks):
                xwt = wpool.tile([65, XW], bf16, tag="xw")
                nc.sync.dma_start(xwt[:], xw_d.ap()[b])
                w2t = wpool.tile([128, O_WLA], bf16, tag="w2")
                nc.sync.dma_start(w2t[:], wb_d.ap()[b][:, 0:O_WLA])
                if b == 0:
                    nc.sync.dma_start(ct[:], ct_d.ap())
                xwts.append(xwt); w2ts.append(w2t)
            for b in range(blocks):
                wlt = wpool.tile([128, WB - O_WLA], bf16, tag="wl")
                nc.sync.dma_start(wlt[:], wb_d.ap()[b][:, O_WLA:WB])
                wlts.append(wlt)

            # PE executes matmuls strictly in program order, so the emission
            # order below software-pipelines the blocks: iteration b emits
            # L1(b), L2(b-1), L3(b-2).  PE then never sits in a tanh round
            # trip — it always has the next block's independent work queued —
            # and the pipeline keeps pace with the DMA stream.
            h1s, h2s, ph2s, pos = {}, {}, {}, {}

            def stage_l1(b):
                xwt = xwts[b]
                xgt = xwt[:, 0:B * C]
                w1tt = xwt[:, B * C:XW]
                # L1: H1preT[256, S] — hidden half t lives at cols
                # [t*512, t*512+S) (512-aligned so no matmul output crosses a
                # PSUM bank boundary).
                ph1 = phpool.tile([128, 1024], f32, tag="ph", name=f"ph1_{b}")
                for j in range(B):
                    for t in range(2):
                        nc.tensor.matmul(
                            ph1[:, t * 512 + j * C : t * 512 + (j + 1) * C],
                            w1tt[:, j * 256 + t * 128 : j * 256 + (t + 1) * 128],
                            xgt[:, j * C : (j + 1) * C],
                        )
                h1 = hpool.tile([128, 2 * S], bf16, tag="h1", name=f"h1_{b}")
                nc.scalar.activation(
                    h1[:].rearrange("p (t s) -> p t s", t=2),
                    ph1[:].rearrange("p (t s) -> p t s", t=2)[:, :, 0:S],
                    Tanh,
                )
                h1s[b] = h1

            def stage_l2(b):
                # L2: bias seeded over the chunk span, then 2 accumulating
                # K-chunks per expert.  The last block is split into expert
                # pairs so its first tanh can start before its last matmul.
                w2t, h1 = w2ts[b], h1s[b]
                w2at = w2t[:, 0:O_W2B]
                w2bt = w2t[:, O_W2B:O_WLA]
                chunks = [(0, B)]
                ph2 = phpool.tile([128, 1024], f32, tag="ph", name=f"ph2_{b}")
                h2 = hpool.tile([128, 2 * S], bf16, tag="h2", name=f"h2_{b}")
                for (j0, j1) in chunks:
                    # bias matmul LAST: every matmul in the group is then
                    # data-gated on h1/w2, so the Tile scheduler cannot hoist
                    # an always-ready bias matmul into the PE wait queue where
                    # its PSUM-slot WAR blocks the whole engine
                    for t in range(2):
                        for j in range(j0, j1):
                            nc.tensor.matmul(
                                ph2[:, t * 512 + j * C : t * 512 + (j + 1) * C],
                                w2at[:, j * 256 + t * 128 : j * 256 + (t + 1) * 128],
                                h1[:, j * C : (j + 1) * C],
                                start=True, stop=False, skip_group_check=True,
                            )
                            nc.tensor.matmul(
                                ph2[:, t * 512 + j * C : t * 512 + (j + 1) * C],
                                w2bt[:, j * 256 + t * 128 : j * 256 + (t + 1) * 128],
                                h1[:, S + j * C : S + (j + 1) * C],
                                start=False, stop=False, skip_group_check=True,
                            )
                        nc.tensor.matmul(
                            ph2[:, t * 512 + j0 * C : t * 512 + j1 * C],
                            w2ct[:, (b * 2 + t) * 128 : (b * 2 + t + 1) * 128],
                            bdt[:, j0 * C : j1 * C],
                            start=False, stop=True, skip_group_check=True,
                        )
                    nc.scalar.activation(
                        h2[:].rearrange("p (t s) -> p t s", t=2)[:, :, j0 * C : j1 * C],
                        ph2[:].rearrange("p (t s) -> p t s", t=2)[:, :, j0 * C : j1 * C],
                        Tanh,
                    )
                ph2s[b], h2s[b] = ph2, h2

            def stage_l3(b):
                # L3: experts packed 2-per-partition-block: expert j -> output
                # partitions [64*(j%2), +64), columns [(j//2)*C, +C).
                wlt, h2 = wlts[b], h2s[b]
                wlat = wlt[:, 0:O_WLB - O_WLA]
                wlbt = wlt[:, O_WLB - O_WLA:WB - O_WLA]
                po = popool.tile([128, ncb * C], f32, tag="po", name=f"po_{b}")
                for j in range(B):
                    h_, cb = j % 2, j // 2
                    nc.tensor.matmul(
                        po[h_ * 64 : (h_ + 1) * 64, cb * C : (cb + 1) * C],
                        wlat[:, j * 64 : (j + 1) * 64],
                        h2[:, j * C : (j + 1) * C],
                        start=True, stop=False, skip_group_check=True,
                    )
                    nc.tensor.matmul(
                        po[h_ * 64 : (h_ + 1) * 64, cb * C : (cb + 1) * C],
                        wlbt[:, j * 64 : (j + 1) * 64],
                        h2[:, S + j * C : S + (j + 1) * C],
                        start=False, stop=False, skip_group_check=True,
                    )
                # bias last (see L2): keeps these matmuls out of the wait
                # queue until the data path is nearly done
                for h in range(2):
                    nc.tensor.matmul(
                        po[h * 64 : (h + 1) * 64, :],
                        wlct[:, (b * 2 + h) * 64 : (b * 2 + h + 1) * 64],
                        bdt[0:ncb, 0 : ncb * C],
                        start=False, stop=True, skip_group_check=True,
                    )
                nc.vector.tensor_copy(
                    ostage[:, b * ncb * C : (b + 1) * ncb * C], po[:]
                )
                if b % 2 == 1:
                    # all stores on the SP queue: its SEQ is free once the
                    # loads are issued, and its DGE delay (650) is the
                    # smallest, which matters for the final store's latency
                    nc.sync.dma_start(
                        out_d.ap()[:, (b - 1) * ncb * C : (b + 1) * ncb * C],
                        ostage[:, (b - 1) * ncb * C : (b + 1) * ncb * C],
                    )

            for b in range(blocks + 1):
                if b < blocks:
                    stage_l1(b)
                if b >= 1:
                    stage_l2(b - 1)
            for b in range(blocks):
                stage_l3(b)

    nc.compile()
    return nc


def _prep_inputs(x, ind, W1, b1, W2, b2, Wl, bl, C, B):
    """Group rows by expert and build the per-core padded device arrays."""
    blocks = EPC // B
    ncb = B // 2
    S = B * C

    order = np.argsort(ind, kind="stable")
    counts = np.bincount(ind, minlength=NEXP)
    offs = np.zeros(NEXP + 1, np.int64)
    np.cumsum(counts, out=offs[1:])
    rows = [order[offs[e]:offs[e + 1]] for e in range(NEXP)]

    # Augmented, transposed weight tables (built once across all cores).
    # W1augT[e] = [65, 256]: rows 0:64 = W1[e].T, row 64 = b1[e].
    w1aug = np.concatenate([W1, b1[:, :, None]], axis=2)  # [E, 256, 65]
    w2aug = np.concatenate([W2, b2[:, :, None]], axis=2)  # [E, 256, 257]
    wlaug = np.concatenate([Wl, bl[:, :, None]], axis=2)  # [E, 64, 257]

    XW = B * (C + 256)
    WB = B * 640
    O_W2B, O_WLA, O_WLB = B * 256, 2 * B * 256, 2 * B * 256 + B * 64
    CT = blocks * 256 + blocks * 128 + S
    O_WLC, O_BD = blocks * 256, blocks * 256 + blocks * 128

    in_maps = []
    for k in range(NCORES):
        es = np.arange(k * EPC, (k + 1) * EPC)
        xw = np.zeros((blocks, 65, XW), np.float32)
        xw[:, 64, 0:B * C] = 1.0  # ones row of the augmented x
        wb = np.empty((blocks, 128, WB), np.float32)
        ct = np.zeros((B, CT), np.float32)
        for b in range(blocks):
            for j in range(B):
                e = es[b * B + j]
                r = rows[e]
                xw[b, 0:64, j * C : j * C + len(r)] = x[r].T
                xw[b, :, B * C + j * 256 : B * C + (j + 1) * 256] = w1aug[e].T
                wb[b, :, j * 256 : (j + 1) * 256] = w2aug[e, :, 0:128].T
                wb[b, :, O_W2B + j * 256 : O_W2B + (j + 1) * 256] = \
                    w2aug[e, :, 128:256].T
                wb[b, :, O_WLA + j * 64 : O_WLA + (j + 1) * 64] = \
                    wlaug[e, :, 0:128].T
                wb[b, :, O_WLB + j * 64 : O_WLB + (j + 1) * 64] = \
                    wlaug[e, :, 128:256].T
                # w2c[j, b, t] = b2-augmented row of expert e, chunk t
                ct[j, b * 256 : (b + 1) * 256] = w2aug[e, :, 256]
            # wlc[cb, b, h] = bl-augmented row of expert B*b + 2*cb + h
            for cb in range(ncb):
                for h in range(2):
                    e = es[b * B + 2 * cb + h]
                    ct[cb, O_WLC + b * 128 + h * 64 : O_WLC + b * 128 + (h + 1) * 64] = \
                        wlaug[e, :, 256]
        for j in range(B):
            ct[j, O_BD + j * C : O_BD + (j + 1) * C] = 1.0
        in_maps.append({
            "xw": xw.astype(BF16),
            "wb": wb.astype(BF16),
            "ct": ct.astype(BF16),
        })
    return in_maps, rows


def _unscatter(results, rows, C, B):
    blocks = EPC // B
    ncb = B // 2
    out = np.empty((N, LIN), np.float32)
    for k in range(NCORES):
        arr = np.asarray(results[k]["out"], np.float32).reshape(2, 64, blocks, ncb, C)
        for b in range(blocks):
            for cb in range(ncb):
                for h in range(2):
                    e = k * EPC + b * B + 2 * cb + h
                    r = rows[e]
                    out[r, :] = arr[h, :, b, cb, 0:len(r)].T
    return out


def kernel(x, ind, W1, b1, W2, b2, Wl, bl):
    from concourse.bass_utils import run_bass_kernel_spmd

    x = np.asarray(x, np.float32)
    ind = np.asarray(ind).astype(np.int64)
    W1 = np.asarray(W1, np.float32); b1 = np.asarray(b1, np.float32)
    W2 = np.asarray(W2, np.float32); b2 = np.asarray(b2, np.float32)
    Wl = np.asarray(Wl, np.float32); bl = np.asarray(bl, np.float32)

    counts = np.bincount(ind, minlength=NEXP)
    C = max(32, int(np.ceil(counts.max() / 32)) * 32)
    assert C <= 256, f"expert load {counts.max()} too imbalanced for this kernel"
    B = 4 if C <= 128 else 2  # keep B*C <= 512 (one PSUM bank per block span)

    key = (C, B)
    if key not in _cache:
        _cache[key] = _build_program(C, B)
    nc = _cache[key]

    in_maps, rows = _prep_inputs(x, ind, W1, b1, W2, b2, Wl, bl, C, B)
    res = run_bass_kernel_spmd(nc, in_maps, core_ids=list(range(NCORES)))
    return _unscatter(res.results, rows, C, B)

